# revision 18
# baseline (speedup 1.0000x reference)
"""Bass/Trainium2 kernel for nn_EnergyModel (3-layer GAT + MLP head).

Sharding: data-parallel over batch B=32 across 8 NeuronCores (4 graphs/core),
GAT/MLP params replicated.

v2 design (per core, G=4 graphs; attention chain runs per PAIR of graphs,
linear stages batched over all 4):
  - atoms_all [c, (g,i)=1024] f32r.
  - hT_all[c, r, (g,i)] = W_r^T atoms (PE, N=512 MMs); bias folded into the
    PSUM evacuation (per-partition tensor_scalar add).
  - h_sb[g][j', ib, (r c)] natural-layout h, bf16 (agg lhsT); bias folded
    into evacuation via tensor_tensor add with broadcast bias tile.
  - src/dst: block-diag selector matmul, batched over graphs (N=512).
  - S blocks per pair: rank-3 matmul with graph-indicator rows:
    lhsT = [dst_g0 | dst_g1 | ones][3, j'], rhs = [ind0 | ind1 | src][3,(g,i)].
  - mask is MULTIPLICATIVE: M = bonds as bf16 {0,1} via SWDGE cast-DMA,
    GPSIMD shuffle to (r,jh,j') order, 2 batched xbar transposes per graph.
  - chain per (pair, r): S 2 MMs -> Prelu (ACT) -> Exp->bf16 (ACT) ->
    *M (GPSIMD) -> agg 4 MMs + Z 2 MMs (PE, bf16 rhs).
  - Z packed in o_ps tile's 2nd bank; rz via reciprocal_approx_fast (DVE);
    broadcast by rank-1 matmul; atoms_next = prelu(o) * rz (DVE).
  - MLP head batched over 4 graphs.
"""

import sys
from contextlib import ExitStack

if "/opt/trn_rl_repo" not in sys.path:
    sys.path.insert(0, "/opt/trn_rl_repo")

import numpy as np

B, N, CIN, C, R, XD = 32, 256, 64, 128, 5, 1024
NCORE = 8
NG = B // NCORE   # graphs per core
NPAIR = NG // 2   # graph pairs per core
NRC = R * C       # 640
H1 = 256          # MLP hidden 1
H2 = 32           # MLP hidden 2
ZDIM = 2 * C + XD  # 1280

_BUILD_CACHE = {}


def build(with_bias=True):
    key = (with_bias,)
    if key in _BUILD_CACHE:
        return _BUILD_CACHE[key]

    import concourse.bass as bass
    from concourse import bacc
    import concourse.tile as tile
    import concourse.mybir as mybir
    from concourse.masks import make_identity

    f32 = mybir.dt.float32
    f32r = mybir.dt.float32r
    bf16 = mybir.dt.bfloat16
    i32 = mybir.dt.int32
    AF = mybir.ActivationFunctionType
    OP = mybir.AluOpType

    nc = bacc.Bacc("TRN2", target_bir_lowering=False)
    mm = nc.tensor.matmul

    atoms_d = nc.dram_tensor("y_atoms", [NG, N, CIN], f32, kind="ExternalInput")
    bonds_d = nc.dram_tensor("y_bonds", [NG, N, N, R], i32, kind="ExternalInput")
    x_d = nc.dram_tensor("x", [NG, XD], f32, kind="ExternalInput")
    W_d = [
        nc.dram_tensor("W1", [CIN, NRC], f32, kind="ExternalInput"),
        nc.dram_tensor("W2", [C, NRC], f32, kind="ExternalInput"),
        nc.dram_tensor("W3", [C, NRC], f32, kind="ExternalInput"),
    ]
    Asd_d = [
        nc.dram_tensor(f"Asd{i}", [CIN if i == 1 else C, 2 * R], f32,
                       kind="ExternalInput")
        for i in (1, 2, 3)
    ]
    We1_d = nc.dram_tensor("We1", [ZDIM, H1], f32, kind="ExternalInput")
    We2_d = nc.dram_tensor("We2", [H1, H2], f32, kind="ExternalInput")
    We3_d = nc.dram_tensor("We3", [H2, 1], f32, kind="ExternalInput")
    if with_bias:
        b_d = [
            nc.dram_tensor(f"b{i}", [1, NRC], f32, kind="ExternalInput")
            for i in (1, 2, 3)
        ]
        sdb_d = nc.dram_tensor("sdb", [2 * R, 1], f32, kind="ExternalInput")
        be1_d = nc.dram_tensor("be1", [1, H1], f32, kind="ExternalInput")
        be2_d = nc.dram_tensor("be2", [1, H2], f32, kind="ExternalInput")
        be3_d = nc.dram_tensor("be3", [1, 1], f32, kind="ExternalInput")
    out_d = nc.dram_tensor("out", [NG, 1], f32, kind="ExternalOutput")

    with tile.TileContext(nc) as tc, ExitStack() as ctx:
        const = ctx.enter_context(tc.tile_pool(name="const", bufs=1))
        gpool = ctx.enter_context(tc.tile_pool(name="gpool", bufs=2))
        spool = ctx.enter_context(tc.tile_pool(name="spool", bufs=2))
        hpool = ctx.enter_context(tc.tile_pool(name="hpool", bufs=1))
        etpool = ctx.enter_context(tc.tile_pool(name="etpool", bufs=3))
        # PSUM: ps_a 1-bank tiles x2 bufs; ps_s 2-bank S chunks x2; ps_o 2-bank x1
        ps_a = ctx.enter_context(tc.tile_pool(name="ps_a", bufs=2, space="PSUM"))
        ps_s = ctx.enter_context(tc.tile_pool(name="ps_s", bufs=2, space="PSUM"))
        ps_o = ctx.enter_context(tc.tile_pool(name="ps_o", bufs=2, space="PSUM"))

        # ---------------- constants ----------------
        ident = const.tile([128, 128], f32)
        make_identity(nc, ident[:])
        ones_col = const.tile([128, 1], bf16)
        nc.vector.memset(ones_col[:], 1.0)
        onesrf = const.tile([1, 128], f32)
        nc.vector.memset(onesrf[:], 1.0)

        W_sb = []
        for li in range(3):
            cin = CIN if li == 0 else C
            w_raw = spool.tile([cin, NRC], f32, tag="w_raw")
            nc.sync.dma_start(w_raw[:], W_d[li][:])
            w = const.tile([cin, NRC], f32r, tag=f"W{li}")
            nc.vector.tensor_copy(w[:], w_raw[:])
            W_sb.append(w)

        # per-layer bias: b_bcast [128, 640] (h natural), b_col [128, 5] (hT)
        if with_bias:
            b_bcast = []
            for li in range(3):
                braw = spool.tile([1, NRC], f32, tag="braw")
                nc.sync.dma_start(braw[:], b_d[li][:])
                bb = const.tile([128, NRC], f32, tag=f"bb{li}")
                nc.gpsimd.partition_broadcast(bb[:], braw[:])
                b_bcast.append(bb)

        # Asd[l] [cin, 10]: host-precomputed (W_r @ a_src[r] | W_r @ a_dst[r])
        # selector so sd = Asd^T @ atoms directly (no hT needed).
        Asd_sb = []
        for li in range(3):
            cin = CIN if li == 0 else C
            asd_raw = spool.tile([cin, 2 * R], f32, tag="asd_raw")
            nc.sync.dma_start(asd_raw[:], Asd_d[li][:])
            asd = const.tile([cin, 2 * R], f32r, tag=f"asd{li}")
            nc.vector.tensor_copy(asd[:], asd_raw[:])
            Asd_sb.append(asd)
        if with_bias:
            sdb_col = const.tile([2 * R, 1], f32)
            nc.sync.dma_start(sdb_col[:], sdb_d[:])

        We1_sb = const.tile([128, 10, H1], f32)
        nc.sync.dma_start(We1_sb[:],
                          We1_d.rearrange("(kb p) n -> p kb n", p=128))
        We2_sb = const.tile([128, 2, H2], f32)
        nc.sync.dma_start(We2_sb[:],
                          We2_d.rearrange("(kb p) n -> p kb n", p=128))
        We3_sb = const.tile([H2, 1], f32)
        nc.sync.dma_start(We3_sb[:], We3_d[:])
        if with_bias:
            be1_row = const.tile([1, H1], f32)
            nc.sync.dma_start(be1_row[:], be1_d[:])
            be2_row = const.tile([1, H2], f32)
            nc.sync.dma_start(be2_row[:], be2_d[:])
            be3_row = const.tile([1, 1], f32)
            nc.sync.dma_start(be3_row[:], be3_d[:])

        # MLP lhsT staging: z^T chunks [128, kb, g]; kb 0..7 = x, 8 = mean, 9 = max
        zT = const.tile([128, 10, NG], f32)

        # Aug tiles for rank-3 S matmuls, one set per pair (layers reuse them
        # serially). srcP: p0/p1 = graph indicators (set once), p2 = src data.
        # dstP: p0/p1 = dst_g0/g1 data, p2 = ones (set once).
        # helper rows (partition 0) DMA'd into non-zero-base partitions:
        # engine writes must start at a 32-aligned partition, DMA writes not.
        ind1_stage = const.tile([1, R, 2 * N], f32r)
        nc.gpsimd.memset(ind1_stage[:].bitcast(f32), 0.0)
        nc.gpsimd.memset(ind1_stage[:, :, N:2 * N].bitcast(f32), 1.0)
        ones_stage = const.tile([1, R, N], f32r)
        nc.gpsimd.memset(ones_stage[:].bitcast(f32), 1.0)
        srcP = []
        dstP = []
        for p in range(NPAIR):
            sP = const.tile([3, R, 2 * N], f32r, tag=f"srcp{p}")
            nc.gpsimd.memset(sP[0:1].bitcast(f32), 0.0)
            nc.gpsimd.memset(sP[0:1, :, 0:N].bitcast(f32), 1.0)
            nc.sync.dma_start(sP[1:2], ind1_stage[:])
            srcP.append(sP)
            dP = const.tile([3, R, N], f32r, tag=f"dstp{p}")
            nc.sync.dma_start(dP[2:3], ones_stage[:])
            dstP.append(dP)

        # ---------------- per-graph preprocessing ----------------
        # masks first: longest dependency chain (bonds DMA -> shuffle ->
        # transpose); overlaps the atoms/params staging below.
        M_T = []
        for p in range(NPAIR):
            M_T.append(const.tile([128, R, 2, 2, 2, 128], bf16, tag=f"mt{p}",
                                  name=f"mt{p}"))
        m_nats = []
        for g in range(NG):
            m_nat = gpool.tile([128, 2, N * R], bf16, tag="mnat", name=f"mnat{g}")
            nc.gpsimd.dma_start(
                m_nat[:],
                bonds_d[g].rearrange("(ib p) j r -> p ib (j r)", ib=2, p=128),
            )
            m_nats.append(m_nat)

        # atoms -> atoms_all [cin, (g, i)] f32r
        atoms_all = gpool.tile([CIN, NG * N], f32r, tag="atoms0")
        for g in range(NG):
            at_nat = spool.tile([128, 2, CIN], f32, tag="atnat")
            for ib in range(2):
                nc.sync.dma_start(at_nat[:, ib, :], atoms_d[g, ib * 128:(ib + 1) * 128, :])
            atT_ps = ps_a.tile([CIN, 2, 128], f32, tag="a")
            for ib in range(2):
                mm(atT_ps[:, ib, :], at_nat[:, ib, :], ident[:],
                   is_transpose=True, start=True, stop=True)
            nc.vector.tensor_copy(
                atoms_all[:, g * N:(g + 1) * N], atT_ps.rearrange("c a b -> c (a b)")
            )

        # mask shuffle (split DVE/ACT) + batched xbar transposes (split SP/ACT)
        for g in range(NG):
            p, gg = divmod(g, 2)
            m_shuf = gpool.tile([128, 2, R, 2, 128], bf16, tag="mshuf")
            for ib in range(2):
                src_ap = m_nats[g][:, ib].rearrange(
                    "p (jh j r) -> p r jh j", jh=2, j=128, r=R)
                if ib == 0:
                    nc.vector.tensor_copy(m_shuf[:, ib], src_ap)
                else:
                    nc.scalar.activation(m_shuf[:, ib], src_ap, AF.Copy)
            for ib in range(2):
                eng = nc.sync if (g * 2 + ib) % 2 == 0 else nc.scalar
                eng.dma_start_transpose(
                    M_T[p][:, :, :, gg, ib, :],
                    m_shuf[:, ib],
                )

        # x staging for MLP
        for g in range(NG):
            x_stage = spool.tile([128, 8], f32, tag="xstage")
            nc.sync.dma_start(x_stage[:], x_d[g].rearrange("(f p) -> p f", p=128))
            nc.vector.tensor_copy(zT[:, 0:8, g:g + 1].rearrange("p a b -> p (a b)"),
                                  x_stage[:])

        # ---------------- GAT layers ----------------
        for li in range(3):
            W = W_sb[li]

            # sd[2r+s, (g,i)] = Asd^T @ atoms (hT folded into Asd on host)
            sd_sb = spool.tile([2 * R, NG * N], f32r, tag="sdsb")
            for h2 in range(2):
                sd_ps = ps_a.tile([2 * R, 512], f32, tag="a")
                mm(sd_ps[:], Asd_sb[li][:],
                   atoms_all[:, h2 * 512:(h2 + 1) * 512],
                   start=True, stop=True)
                dst = sd_sb[:, h2 * 512:(h2 + 1) * 512]
                if with_bias:
                    nc.vector.tensor_scalar(
                        dst, sd_ps[:], sdb_col[:], None, op0=OP.add,
                    )
                else:
                    nc.vector.tensor_copy(dst, sd_ps[:])

            # scatter src/dst into per-pair aug tiles
            for p in range(NPAIR):
                nc.gpsimd.dma_start(
                    srcP[p][2:3],
                    sd_sb[0:R, p * 512:(p + 1) * 512],
                )
                for gg in range(2):
                    g = 2 * p + gg
                    nc.gpsimd.dma_start(
                        dstP[p][gg:gg + 1, :, :],
                        sd_sb[R:2 * R, g * N:(g + 1) * N],
                    )

            # h natural per graph [j', ib, (r c)] bf16
            h_sb = []
            for g in range(NG):
                hs = hpool.tile([128, 2, NRC], bf16, tag=f"h{g}")
                for ib in range(2):
                    lt = atoms_all[:, (2 * g + ib) * 128:(2 * g + ib + 1) * 128]
                    hA = ps_a.tile([128, 384], f32, tag="a")
                    mm(hA[:], lt, W[:, 0:384], start=True, stop=True)
                    hB = ps_a.tile([128, 256], f32, tag="a")
                    mm(hB[:], lt, W[:, 384:NRC], start=True, stop=True)
                    if with_bias:
                        nc.vector.tensor_tensor(
                            hs[:, ib, 0:384], hA[:], b_bcast[li][:, 0:384], op=OP.add
                        )
                        nc.vector.tensor_tensor(
                            hs[:, ib, 384:NRC], hB[:], b_bcast[li][:, 384:NRC], op=OP.add
                        )
                    else:
                        nc.scalar.activation(hs[:, ib, 0:384], hA[:], AF.Copy)
                        nc.scalar.activation(hs[:, ib, 384:NRC], hB[:], AF.Copy)
                h_sb.append(hs)

            # ---- attention chain, both pairs interleaved per (r, jh) block ----
            atoms_next = gpool.tile([C, NG * N], f32r, tag="atoms_n")
            # o_ps[p]: [:, 0, :] = aggregation out; [0:1, 1, :] = Z row
            o_ps = [ps_o.tile([C, 2, 2 * N], f32, tag="o", name=f"o{p}")
                    for p in range(NPAIR)]
            for r in range(R):
                for p in range(NPAIR):
                    for jh in range(2):
                        S_ps = ps_s.tile([128, 2 * N], f32, tag="s")
                        mm(S_ps[:],
                           dstP[p][:, r, jh * 128:(jh + 1) * 128],
                           srcP[p][:, r, :],
                           start=True, stop=True)
                        L_sb = etpool.tile([128, 2 * N], f32, tag="l", bufs=2)
                        nc.scalar.activation(L_sb[:], S_ps[:], AF.Prelu, alpha=0.2)
                        E_sb = etpool.tile([128, 2 * N], bf16, tag="e", bufs=2)
                        nc.scalar.activation(E_sb[:], L_sb[:], AF.Exp)
                        Et = etpool.tile([128, 2 * N], bf16, tag="et", bufs=4)
                        nc.vector.tensor_tensor(
                            Et[:], E_sb[:],
                            M_T[p][:, r, jh].rearrange("p a b c -> p (a b c)"),
                            op=OP.mult,
                        )
                        # gg0 and gg1 accumulate in the SAME PSUM bank: only
                        # the very first matmul carries start=True (it clears
                        # the whole bank's has_written bits); gg1 joins with
                        # start=False on the freshly cleared bank.
                        for gg in range(2):
                            g = 2 * p + gg
                            mm(o_ps[p][:, 0, gg * N:(gg + 1) * N],
                               h_sb[g][:, jh, r * C:(r + 1) * C],
                               Et[:, gg * N:(gg + 1) * N],
                               start=(r == 0 and jh == 0 and gg == 0),
                               stop=(r == R - 1 and jh == 1),
                               skip_group_check=True)
                        mm(o_ps[p][0:1, 1, :], ones_col[:], Et[:],
                           start=(r == 0 and jh == 0),
                           stop=(r == R - 1 and jh == 1))

            # normalize: rz = 1/Z; broadcast; atoms_next = prelu(o) * rz
            for p in range(NPAIR):
                rz_sb = spool.tile([1, 2 * N], f32, tag="rz")
                nc.vector.reciprocal_approx_fast(rz_sb[:], o_ps[p][0:1, 1, :])
                O_sb = spool.tile([C, 2 * N], f32, tag="osb")
                if li < 2:
                    nc.scalar.activation(O_sb[:], o_ps[p][:, 0, :], AF.Prelu, alpha=0.2)
                else:
                    nc.scalar.activation(O_sb[:], o_ps[p][:, 0, :], AF.Copy)
                rzb_sb = spool.tile([128, 2 * N], f32, tag="rzb")
                nc.gpsimd.partition_broadcast(rzb_sb[:], rz_sb[:])
                nc.vector.tensor_tensor(
                    atoms_next[:, p * 512:(p + 1) * 512], O_sb[:], rzb_sb[:],
                    op=OP.mult,
                )
            atoms_all = atoms_next

        # ---------------- y_feats + MLP head ----------------
        for g in range(NG):
            h3 = atoms_all[:, g * N:(g + 1) * N]
            mean_raw = spool.tile([128, 1], f32, tag="mean")
            nc.vector.tensor_reduce(mean_raw[:], h3, axis=mybir.AxisListType.X,
                                    op=OP.add)
            nc.vector.tensor_scalar(zT[:, 8, g:g + 1], mean_raw[:], 1.0 / N, None,
                                    op0=OP.mult)
            nc.vector.tensor_reduce(zT[:, 9, g:g + 1], h3, axis=mybir.AxisListType.X,
                                    op=OP.max)

        zz_ps = ps_a.tile([NG, H1], f32, tag="a")
        for kb in range(10):
            mm(zz_ps[:], zT[:, kb, :], We1_sb[:, kb, :],
               start=(kb == 0), stop=(kb == 9) and not with_bias)
        if with_bias:
            mm(zz_ps[:], onesrf[:, :NG], be1_row[:], start=False, stop=True)
        zzl = spool.tile([NG, H1], f32, tag="zzl")
        nc.scalar.activation(zzl[:], zz_ps[:], AF.Prelu, alpha=0.2)
        zzT_ps = ps_a.tile([128, 2, NG], f32, tag="a")
        for hh in range(2):
            mm(zzT_ps[:, hh, :], zzl[:, hh * 128:(hh + 1) * 128],
               ident[:NG, :NG], is_transpose=True, start=True, stop=True)
        zzT_sb = spool.tile([128, 2, NG], f32, tag="zzt")
        nc.vector.tensor_copy(zzT_sb[:], zzT_ps[:])

        z2_ps = ps_a.tile([NG, H2], f32, tag="a")
        for hh in range(2):
            mm(z2_ps[:], zzT_sb[:, hh, :], We2_sb[:, hh, :],
               start=(hh == 0), stop=(hh == 1) and not with_bias)
        if with_bias:
            mm(z2_ps[:], onesrf[:, :NG], be2_row[:], start=False, stop=True)
        z2l = spool.tile([NG, H2], f32, tag="z2l")
        nc.scalar.activation(z2l[:], z2_ps[:], AF.Prelu, alpha=0.2)
        z2T_ps = ps_a.tile([H2, NG], f32, tag="a")
        mm(z2T_ps[:], z2l[:], ident[:NG, :NG], is_transpose=True,
           start=True, stop=True)
        z2T_sb = spool.tile([H2, NG], f32, tag="z2t")
        nc.vector.tensor_copy(z2T_sb[:], z2T_ps[:])

        y_ps = ps_a.tile([NG, 1], f32, tag="a")
        mm(y_ps[:], z2T_sb[:], We3_sb[:], start=True, stop=not with_bias)
        if with_bias:
            mm(y_ps[:], onesrf[:, :NG], be3_row[:], start=False, stop=True)
        y_sb = spool.tile([NG, 1], f32, tag="y")
        nc.vector.tensor_copy(y_sb[:], y_ps[:])
        nc.sync.dma_start(out_d[:], y_sb[:])

    nc.compile()
    _BUILD_CACHE[key] = nc
    return nc


_PARAM_KEYS = ("W1", "W2", "W3", "We1", "We2", "We3")
_BIAS_KEYS = ("b1", "b2", "b3", "be1", "be2", "be3")


def _derived_params(inputs, with_bias):
    # Asd[li] [cin, 2R]: col r = W_r @ a_src[r], col R+r = W_r @ a_dst[r]
    # so that sd = Asd^T @ atomsT gives [src rows 0..R-1 | dst rows R..2R-1].
    d = {}
    sdb = np.zeros((2 * R, 1), np.float32)
    for li, (wk, ak, bk) in enumerate(
        (("W1", "a1", "b1"), ("W2", "a2", "b2"), ("W3", "a3", "b3"))
    ):
        W = np.asarray(inputs[wk], np.float32)
        cin = W.shape[0]
        Wr = W.reshape(cin, R, C)
        a = np.asarray(inputs[ak], np.float32)
        asd = np.zeros((cin, 2 * R), np.float32)
        for r in range(R):
            asd[:, r] = Wr[:, r, :] @ a[r, :C]
            asd[:, R + r] = Wr[:, r, :] @ a[r, C:]
        d[f"Asd{li + 1}"] = asd
        if with_bias:
            b = np.asarray(inputs[bk], np.float32).reshape(R, C)
            if li == 0:
                for r in range(R):
                    sdb[r, 0] = b[r] @ a[r, :C]
                    sdb[R + r, 0] = b[r] @ a[r, C:]
    if with_bias:
        d["sdb"] = sdb
    return d


def _shard_inputs(inputs, with_bias, n_cores, ng):
    derived = _derived_params(inputs, with_bias)
    per_core = []
    for c in range(n_cores):
        s = slice(c * ng, (c + 1) * ng)
        m = {
            "y_atoms": np.ascontiguousarray(inputs["y_atoms"][s], np.float32),
            "y_bonds": np.ascontiguousarray(inputs["y_bonds"][s], np.int32),
            "x": np.ascontiguousarray(inputs["x"][s], np.float32),
        }
        for k in _PARAM_KEYS:
            m[k] = np.ascontiguousarray(inputs[k], np.float32)
        for k, v in derived.items():
            m[k] = np.ascontiguousarray(v, np.float32)
        if with_bias:
            for k in _BIAS_KEYS:
                m[k] = np.ascontiguousarray(np.asarray(inputs[k], np.float32).reshape(1, -1))
        per_core.append(m)
    return per_core


def _needs_bias(inputs):
    return any(np.abs(np.asarray(inputs[k])).max() > 0 for k in _BIAS_KEYS)


def kernel(**inputs):
    from concourse.bass_utils import run_bass_kernel_spmd

    with_bias = _needs_bias(inputs)
    nc = build(with_bias)
    in_maps = _shard_inputs(inputs, with_bias, NCORE, NG)
    res = run_bass_kernel_spmd(nc, in_maps, core_ids=list(range(NCORE)))
    out = np.concatenate([r["out"] for r in res.results], axis=0)
    return np.ascontiguousarray(out, np.float32)


# revision 20
# speedup vs baseline: 1.0674x; 1.0674x over previous
"""Bass/Trainium2 kernel for nn_EnergyModel (3-layer GAT + MLP head).

Sharding: data-parallel over batch B=32 across 8 NeuronCores (4 graphs/core),
GAT/MLP params replicated.

v2 design (per core, G=4 graphs; attention chain runs per PAIR of graphs,
linear stages batched over all 4):
  - atoms_all [c, (g,i)=1024] f32r.
  - hT_all[c, r, (g,i)] = W_r^T atoms (PE, N=512 MMs); bias folded into the
    PSUM evacuation (per-partition tensor_scalar add).
  - h_sb[g][j', ib, (r c)] natural-layout h, bf16 (agg lhsT); bias folded
    into evacuation via tensor_tensor add with broadcast bias tile.
  - src/dst: block-diag selector matmul, batched over graphs (N=512).
  - S blocks per pair: rank-3 matmul with graph-indicator rows:
    lhsT = [dst_g0 | dst_g1 | ones][3, j'], rhs = [ind0 | ind1 | src][3,(g,i)].
  - mask is MULTIPLICATIVE: M = bonds as bf16 {0,1} via SWDGE cast-DMA,
    GPSIMD shuffle to (r,jh,j') order, 2 batched xbar transposes per graph.
  - chain per (pair, r): S 2 MMs -> Prelu (ACT) -> Exp->bf16 (ACT) ->
    *M (GPSIMD) -> agg 4 MMs + Z 2 MMs (PE, bf16 rhs).
  - Z packed in o_ps tile's 2nd bank; rz via reciprocal_approx_fast (DVE);
    broadcast by rank-1 matmul; atoms_next = prelu(o) * rz (DVE).
  - MLP head batched over 4 graphs.
"""

import sys
from contextlib import ExitStack

if "/opt/trn_rl_repo" not in sys.path:
    sys.path.insert(0, "/opt/trn_rl_repo")

import numpy as np

B, N, CIN, C, R, XD = 32, 256, 64, 128, 5, 1024
NCORE = 8
NG = B // NCORE   # graphs per core
NPAIR = NG // 2   # graph pairs per core
NRC = R * C       # 640
H1 = 256          # MLP hidden 1
H2 = 32           # MLP hidden 2
ZDIM = 2 * C + XD  # 1280

_BUILD_CACHE = {}


def build(with_bias=True):
    key = (with_bias,)
    if key in _BUILD_CACHE:
        return _BUILD_CACHE[key]

    import concourse.bass as bass
    from concourse import bacc
    import concourse.tile as tile
    import concourse.mybir as mybir
    from concourse.masks import make_identity

    f32 = mybir.dt.float32
    f32r = mybir.dt.float32r
    bf16 = mybir.dt.bfloat16
    i32 = mybir.dt.int32
    AF = mybir.ActivationFunctionType
    OP = mybir.AluOpType

    nc = bacc.Bacc("TRN2", target_bir_lowering=False)
    mm = nc.tensor.matmul

    atoms_d = nc.dram_tensor("y_atoms", [NG, N, CIN], f32, kind="ExternalInput")
    bonds_d = nc.dram_tensor("y_bonds", [NG, N, N, R], i32, kind="ExternalInput")
    x_d = nc.dram_tensor("x", [NG, XD], f32, kind="ExternalInput")
    W_d = [
        nc.dram_tensor("W1", [CIN, NRC], f32, kind="ExternalInput"),
        nc.dram_tensor("W2", [C, NRC], f32, kind="ExternalInput"),
        nc.dram_tensor("W3", [C, NRC], f32, kind="ExternalInput"),
    ]
    aug_d = nc.dram_tensor("aug_init", [3, R, 2 * N], f32, kind="ExternalInput")
    Asd_d = [
        nc.dram_tensor(f"Asd{i}", [CIN if i == 1 else C, 2 * R], f32,
                       kind="ExternalInput")
        for i in (1, 2, 3)
    ]
    We1_d = nc.dram_tensor("We1", [ZDIM, H1], f32, kind="ExternalInput")
    We2_d = nc.dram_tensor("We2", [H1, H2], f32, kind="ExternalInput")
    We3_d = nc.dram_tensor("We3", [H2, 1], f32, kind="ExternalInput")
    if with_bias:
        b_d = [
            nc.dram_tensor(f"b{i}", [1, NRC], f32, kind="ExternalInput")
            for i in (1, 2, 3)
        ]
        sdb_d = nc.dram_tensor("sdb", [2 * R, 1], f32, kind="ExternalInput")
        be1_d = nc.dram_tensor("be1", [1, H1], f32, kind="ExternalInput")
        be2_d = nc.dram_tensor("be2", [1, H2], f32, kind="ExternalInput")
        be3_d = nc.dram_tensor("be3", [1, 1], f32, kind="ExternalInput")
    out_d = nc.dram_tensor("out", [NG, 1], f32, kind="ExternalOutput")

    with tile.TileContext(nc) as tc, ExitStack() as ctx:
        const = ctx.enter_context(tc.tile_pool(name="const", bufs=1))
        gpool = ctx.enter_context(tc.tile_pool(name="gpool", bufs=2))
        spool = ctx.enter_context(tc.tile_pool(name="spool", bufs=2))
        hpool = ctx.enter_context(tc.tile_pool(name="hpool", bufs=1))
        etpool = ctx.enter_context(tc.tile_pool(name="etpool", bufs=3))
        # PSUM: ps_a 1-bank tiles x2 bufs; ps_s 2-bank S chunks x2; ps_o 2-bank x1
        ps_a = ctx.enter_context(tc.tile_pool(name="ps_a", bufs=2, space="PSUM"))
        ps_s = ctx.enter_context(tc.tile_pool(name="ps_s", bufs=2, space="PSUM"))
        ps_o = ctx.enter_context(tc.tile_pool(name="ps_o", bufs=2, space="PSUM"))

        # ---------------- constants ----------------
        ident = const.tile([128, 128], f32)
        make_identity(nc, ident[:])
        ones_col = const.tile([128, 1], bf16)
        nc.vector.memset(ones_col[:], 1.0)
        onesrf = const.tile([1, 128], f32)
        nc.vector.memset(onesrf[:], 1.0)

        W_sb = []
        for li in range(3):
            cin = CIN if li == 0 else C
            w_raw = spool.tile([cin, NRC], f32, tag="w_raw")
            nc.sync.dma_start(w_raw[:], W_d[li][:])
            w = const.tile([cin, NRC], f32r, tag=f"W{li}")
            nc.vector.tensor_copy(w[:], w_raw[:])
            W_sb.append(w)

        # per-layer bias: b_bcast [128, 640] (h natural), b_col [128, 5] (hT)
        if with_bias:
            b_bcast = []
            for li in range(3):
                braw = spool.tile([1, NRC], f32, tag="braw")
                nc.sync.dma_start(braw[:], b_d[li][:])
                bb = const.tile([128, NRC], f32, tag=f"bb{li}")
                nc.gpsimd.partition_broadcast(bb[:], braw[:])
                b_bcast.append(bb)

        # Asd[l] [cin, 10]: host-precomputed (W_r @ a_src[r] | W_r @ a_dst[r])
        # selector so sd = Asd^T @ atoms directly (no hT needed).
        Asd_sb = []
        for li in range(3):
            cin = CIN if li == 0 else C
            asd_raw = spool.tile([cin, 2 * R], f32, tag="asd_raw")
            nc.sync.dma_start(asd_raw[:], Asd_d[li][:])
            asd = const.tile([cin, 2 * R], f32r, tag=f"asd{li}")
            nc.vector.tensor_copy(asd[:], asd_raw[:])
            Asd_sb.append(asd)
        if with_bias:
            sdb_col = const.tile([2 * R, 1], f32)
            nc.sync.dma_start(sdb_col[:], sdb_d[:])

        We1_sb = const.tile([128, 10, H1], f32)
        nc.sync.dma_start(We1_sb[:],
                          We1_d.rearrange("(kb p) n -> p kb n", p=128))
        We2_sb = const.tile([128, 2, H2], f32)
        nc.sync.dma_start(We2_sb[:],
                          We2_d.rearrange("(kb p) n -> p kb n", p=128))
        We3_sb = const.tile([H2, 1], f32)
        nc.sync.dma_start(We3_sb[:], We3_d[:])
        if with_bias:
            be1_row = const.tile([1, H1], f32)
            nc.sync.dma_start(be1_row[:], be1_d[:])
            be2_row = const.tile([1, H2], f32)
            nc.sync.dma_start(be2_row[:], be2_d[:])
            be3_row = const.tile([1, 1], f32)
            nc.sync.dma_start(be3_row[:], be3_d[:])

        # MLP lhsT staging: z^T chunks [128, kb, g]; kb 0..7 = x, 8 = mean, 9 = max
        zT = const.tile([128, 10, NG], f32)

        # Aug tiles for rank-3 S matmuls, one set per pair (layers reuse them
        # serially). srcP: p0/p1 = graph indicators, p2 = src data.
        # dstP: p0/p1 = dst_g0/g1 data, p2 = ones. Indicator/ones patterns come
        # from the host-built aug_init tensor (exact 0/1 -> f32r-safe).
        aug_stage = spool.tile([3, R, 2 * N], f32, tag="augst")
        nc.sync.dma_start(aug_stage[:], aug_d[:])
        aug_sb = const.tile([3, R, 2 * N], f32r)
        nc.vector.tensor_copy(aug_sb[:], aug_stage[:])
        srcP = []
        dstP = []
        for p in range(NPAIR):
            sP = const.tile([3, R, 2 * N], f32r, tag=f"srcp{p}")
            nc.sync.dma_start(sP[0:2], aug_sb[0:2])
            srcP.append(sP)
            dP = const.tile([3, R, N], f32r, tag=f"dstp{p}")
            nc.sync.dma_start(dP[2:3], aug_sb[2:3, :, 0:N])
            dstP.append(dP)

        # ---------------- per-graph preprocessing ----------------
        # masks first: bonds -> bf16 {0,1} via SWDGE cast-DMA, then PE
        # transpose-mode matmuls reading the strided (jh,j',r) layout
        # directly; evacuated into M_T[pair] [j', r, jh, g, ib, i'].
        M_T = []
        for p in range(NPAIR):
            M_T.append(const.tile([128, R, 2, 2, 2, 128], bf16, tag=f"mt{p}",
                                  name=f"mt{p}"))
        m_nats = []
        for g in range(NG):
            m_nat = gpool.tile([128, 2, 2, 128, R], bf16, tag="mnat", name=f"mnat{g}")
            nc.gpsimd.dma_start(
                m_nat.rearrange("p ib jh j r -> p ib (jh j r)"),
                bonds_d[g].rearrange("(ib p) j r -> p ib (j r)", ib=2, p=128),
            )
            m_nats.append(m_nat)

        # atoms -> atoms_all [cin, (g, i)] f32r
        atoms_all = gpool.tile([CIN, NG * N], f32r, tag="atoms0")
        for g in range(NG):
            at_nat = spool.tile([128, 2, CIN], f32, tag="atnat")
            for ib in range(2):
                nc.sync.dma_start(at_nat[:, ib, :], atoms_d[g, ib * 128:(ib + 1) * 128, :])
            atT_ps = ps_a.tile([CIN, 2, 128], f32, tag="a")
            for ib in range(2):
                mm(atT_ps[:, ib, :], at_nat[:, ib, :], ident[:],
                   is_transpose=True, start=True, stop=True)
            nc.vector.tensor_copy(
                atoms_all[:, g * N:(g + 1) * N], atT_ps.rearrange("c a b -> c (a b)")
            )

        # mask transposes on PE (idle at startup; avoids the xbar-DMA
        # serialization against regular DMAs)
        ident_bf = const.tile([128, 128], bf16)
        nc.vector.tensor_copy(ident_bf[:], ident[:])
        for g in range(NG):
            p, gg = divmod(g, 2)
            for ib in range(2):
                for rg, nb in ((0, 4), (4, 4), (8, 2)):
                    mt_ps = ps_a.tile([128, 4, 128], bf16, tag="a")
                    for k in range(nb):
                        b = rg + k
                        r, jh = b // 2, b % 2
                        mm(mt_ps[:, k, :], m_nats[g][:, ib, jh, :, r],
                           ident_bf[:], is_transpose=True, start=True, stop=True)
                    nc.vector.tensor_copy(
                        M_T[p].rearrange("p r jh g ib i -> p (r jh) g ib i")[
                            :, rg:rg + nb, gg, ib, :],
                        mt_ps[:, 0:nb, :],
                    )

        # x staging for MLP
        for g in range(NG):
            x_stage = spool.tile([128, 8], f32, tag="xstage")
            nc.sync.dma_start(x_stage[:], x_d[g].rearrange("(f p) -> p f", p=128))
            nc.vector.tensor_copy(zT[:, 0:8, g:g + 1].rearrange("p a b -> p (a b)"),
                                  x_stage[:])

        # ---------------- GAT layers ----------------
        for li in range(3):
            W = W_sb[li]

            # sd[2r+s, (g,i)] = Asd^T @ atoms (hT folded into Asd on host)
            sd_sb = spool.tile([2 * R, NG * N], f32r, tag="sdsb")
            for h2 in range(2):
                sd_ps = ps_a.tile([2 * R, 512], f32, tag="a")
                mm(sd_ps[:], Asd_sb[li][:],
                   atoms_all[:, h2 * 512:(h2 + 1) * 512],
                   start=True, stop=True)
                dst = sd_sb[:, h2 * 512:(h2 + 1) * 512]
                if with_bias:
                    nc.vector.tensor_scalar(
                        dst, sd_ps[:], sdb_col[:], None, op0=OP.add,
                    )
                else:
                    nc.vector.tensor_copy(dst, sd_ps[:])

            # scatter src/dst into per-pair aug tiles
            for p in range(NPAIR):
                nc.gpsimd.dma_start(
                    srcP[p][2:3],
                    sd_sb[0:R, p * 512:(p + 1) * 512],
                )
                for gg in range(2):
                    g = 2 * p + gg
                    nc.gpsimd.dma_start(
                        dstP[p][gg:gg + 1, :, :],
                        sd_sb[R:2 * R, g * N:(g + 1) * N],
                    )

            # h natural per graph [j', ib, (r c)] bf16
            h_sb = []
            for g in range(NG):
                hs = hpool.tile([128, 2, NRC], bf16, tag=f"h{g}")
                for ib in range(2):
                    lt = atoms_all[:, (2 * g + ib) * 128:(2 * g + ib + 1) * 128]
                    hA = ps_a.tile([128, 384], f32, tag="a")
                    mm(hA[:], lt, W[:, 0:384], start=True, stop=True)
                    hB = ps_a.tile([128, 256], f32, tag="a")
                    mm(hB[:], lt, W[:, 384:NRC], start=True, stop=True)
                    if with_bias:
                        nc.vector.tensor_tensor(
                            hs[:, ib, 0:384], hA[:], b_bcast[li][:, 0:384], op=OP.add
                        )
                        nc.vector.tensor_tensor(
                            hs[:, ib, 384:NRC], hB[:], b_bcast[li][:, 384:NRC], op=OP.add
                        )
                    else:
                        nc.scalar.activation(hs[:, ib, 0:384], hA[:], AF.Copy)
                        nc.scalar.activation(hs[:, ib, 384:NRC], hB[:], AF.Copy)
                h_sb.append(hs)

            # ---- attention chain, both pairs interleaved per (r, jh) block ----
            atoms_next = gpool.tile([C, NG * N], f32r, tag="atoms_n")
            # o_ps[p]: [:, 0, :] = aggregation out; [0:1, 1, :] = Z row
            o_ps = [ps_o.tile([C, 2, 2 * N], f32, tag="o", name=f"o{p}")
                    for p in range(NPAIR)]
            for r in range(R):
                for p in range(NPAIR):
                    for jh in range(2):
                        S_ps = ps_s.tile([128, 2 * N], f32, tag="s")
                        mm(S_ps[:],
                           dstP[p][:, r, jh * 128:(jh + 1) * 128],
                           srcP[p][:, r, :],
                           start=True, stop=True)
                        L_sb = etpool.tile([128, 2 * N], f32, tag="l", bufs=2)
                        nc.scalar.activation(L_sb[:], S_ps[:], AF.Prelu, alpha=0.2)
                        E_sb = etpool.tile([128, 2 * N], bf16, tag="e", bufs=2)
                        nc.scalar.activation(E_sb[:], L_sb[:], AF.Exp)
                        Et = etpool.tile([128, 2 * N], bf16, tag="et", bufs=4)
                        nc.vector.tensor_tensor(
                            Et[:], E_sb[:],
                            M_T[p][:, r, jh].rearrange("p a b c -> p (a b c)"),
                            op=OP.mult,
                        )
                        # gg0 and gg1 accumulate in the SAME PSUM bank: only
                        # the very first matmul carries start=True (it clears
                        # the whole bank's has_written bits); gg1 joins with
                        # start=False on the freshly cleared bank.
                        for gg in range(2):
                            g = 2 * p + gg
                            mm(o_ps[p][:, 0, gg * N:(gg + 1) * N],
                               h_sb[g][:, jh, r * C:(r + 1) * C],
                               Et[:, gg * N:(gg + 1) * N],
                               start=(r == 0 and jh == 0 and gg == 0),
                               stop=(r == R - 1 and jh == 1),
                               skip_group_check=True)
                        mm(o_ps[p][0:1, 1, :], ones_col[:], Et[:],
                           start=(r == 0 and jh == 0),
                           stop=(r == R - 1 and jh == 1))

            # normalize: rz = 1/Z; broadcast; atoms_next = prelu(o) * rz
            for p in range(NPAIR):
                rz_sb = spool.tile([1, 2 * N], f32, tag="rz")
                nc.vector.reciprocal_approx_fast(rz_sb[:], o_ps[p][0:1, 1, :])
                O_sb = spool.tile([C, 2 * N], f32, tag="osb")
                if li < 2:
                    nc.scalar.activation(O_sb[:], o_ps[p][:, 0, :], AF.Prelu, alpha=0.2)
                else:
                    nc.scalar.activation(O_sb[:], o_ps[p][:, 0, :], AF.Copy)
                rzb_sb = spool.tile([128, 2 * N], f32, tag="rzb")
                nc.gpsimd.partition_broadcast(rzb_sb[:], rz_sb[:])
                nc.vector.tensor_tensor(
                    atoms_next[:, p * 512:(p + 1) * 512], O_sb[:], rzb_sb[:],
                    op=OP.mult,
                )
            atoms_all = atoms_next

        # ---------------- y_feats + MLP head ----------------
        for g in range(NG):
            h3 = atoms_all[:, g * N:(g + 1) * N]
            mean_raw = spool.tile([128, 1], f32, tag="mean")
            nc.vector.tensor_reduce(mean_raw[:], h3, axis=mybir.AxisListType.X,
                                    op=OP.add)
            nc.vector.tensor_scalar(zT[:, 8, g:g + 1], mean_raw[:], 1.0 / N, None,
                                    op0=OP.mult)
            nc.vector.tensor_reduce(zT[:, 9, g:g + 1], h3, axis=mybir.AxisListType.X,
                                    op=OP.max)

        zz_ps = ps_a.tile([NG, H1], f32, tag="a")
        for kb in range(10):
            mm(zz_ps[:], zT[:, kb, :], We1_sb[:, kb, :],
               start=(kb == 0), stop=(kb == 9) and not with_bias)
        if with_bias:
            mm(zz_ps[:], onesrf[:, :NG], be1_row[:], start=False, stop=True)
        zzl = spool.tile([NG, H1], f32, tag="zzl")
        nc.scalar.activation(zzl[:], zz_ps[:], AF.Prelu, alpha=0.2)
        zzT_ps = ps_a.tile([128, 2, NG], f32, tag="a")
        for hh in range(2):
            mm(zzT_ps[:, hh, :], zzl[:, hh * 128:(hh + 1) * 128],
               ident[:NG, :NG], is_transpose=True, start=True, stop=True)
        zzT_sb = spool.tile([128, 2, NG], f32, tag="zzt")
        nc.vector.tensor_copy(zzT_sb[:], zzT_ps[:])

        z2_ps = ps_a.tile([NG, H2], f32, tag="a")
        for hh in range(2):
            mm(z2_ps[:], zzT_sb[:, hh, :], We2_sb[:, hh, :],
               start=(hh == 0), stop=(hh == 1) and not with_bias)
        if with_bias:
            mm(z2_ps[:], onesrf[:, :NG], be2_row[:], start=False, stop=True)
        z2l = spool.tile([NG, H2], f32, tag="z2l")
        nc.scalar.activation(z2l[:], z2_ps[:], AF.Prelu, alpha=0.2)
        z2T_ps = ps_a.tile([H2, NG], f32, tag="a")
        mm(z2T_ps[:], z2l[:], ident[:NG, :NG], is_transpose=True,
           start=True, stop=True)
        z2T_sb = spool.tile([H2, NG], f32, tag="z2t")
        nc.vector.tensor_copy(z2T_sb[:], z2T_ps[:])

        y_ps = ps_a.tile([NG, 1], f32, tag="a")
        mm(y_ps[:], z2T_sb[:], We3_sb[:], start=True, stop=not with_bias)
        if with_bias:
            mm(y_ps[:], onesrf[:, :NG], be3_row[:], start=False, stop=True)
        y_sb = spool.tile([NG, 1], f32, tag="y")
        nc.vector.tensor_copy(y_sb[:], y_ps[:])
        nc.sync.dma_start(out_d[:], y_sb[:])

    nc.compile()
    _BUILD_CACHE[key] = nc
    return nc


_PARAM_KEYS = ("W1", "W2", "W3", "We1", "We2", "We3")
_BIAS_KEYS = ("b1", "b2", "b3", "be1", "be2", "be3")


def _derived_params(inputs, with_bias):
    # Asd[li] [cin, 2R]: col r = W_r @ a_src[r], col R+r = W_r @ a_dst[r]
    # so that sd = Asd^T @ atomsT gives [src rows 0..R-1 | dst rows R..2R-1].
    d = {}
    aug = np.zeros((3, R, 2 * N), np.float32)
    aug[0, :, 0:N] = 1.0
    aug[1, :, N:2 * N] = 1.0
    aug[2] = 1.0
    d["aug_init"] = aug
    sdb = np.zeros((2 * R, 1), np.float32)
    for li, (wk, ak, bk) in enumerate(
        (("W1", "a1", "b1"), ("W2", "a2", "b2"), ("W3", "a3", "b3"))
    ):
        W = np.asarray(inputs[wk], np.float32)
        cin = W.shape[0]
        Wr = W.reshape(cin, R, C)
        a = np.asarray(inputs[ak], np.float32)
        asd = np.zeros((cin, 2 * R), np.float32)
        for r in range(R):
            asd[:, r] = Wr[:, r, :] @ a[r, :C]
            asd[:, R + r] = Wr[:, r, :] @ a[r, C:]
        d[f"Asd{li + 1}"] = asd
        if with_bias:
            b = np.asarray(inputs[bk], np.float32).reshape(R, C)
            if li == 0:
                for r in range(R):
                    sdb[r, 0] = b[r] @ a[r, :C]
                    sdb[R + r, 0] = b[r] @ a[r, C:]
    if with_bias:
        d["sdb"] = sdb
    return d


def _shard_inputs(inputs, with_bias, n_cores, ng):
    derived = _derived_params(inputs, with_bias)
    per_core = []
    for c in range(n_cores):
        s = slice(c * ng, (c + 1) * ng)
        m = {
            "y_atoms": np.ascontiguousarray(inputs["y_atoms"][s], np.float32),
            "y_bonds": np.ascontiguousarray(inputs["y_bonds"][s], np.int32),
            "x": np.ascontiguousarray(inputs["x"][s], np.float32),
        }
        for k in _PARAM_KEYS:
            m[k] = np.ascontiguousarray(inputs[k], np.float32)
        for k, v in derived.items():
            m[k] = np.ascontiguousarray(v, np.float32)
        if with_bias:
            for k in _BIAS_KEYS:
                m[k] = np.ascontiguousarray(np.asarray(inputs[k], np.float32).reshape(1, -1))
        per_core.append(m)
    return per_core


def _needs_bias(inputs):
    return any(np.abs(np.asarray(inputs[k])).max() > 0 for k in _BIAS_KEYS)


def kernel(**inputs):
    from concourse.bass_utils import run_bass_kernel_spmd

    with_bias = _needs_bias(inputs)
    nc = build(with_bias)
    in_maps = _shard_inputs(inputs, with_bias, NCORE, NG)
    res = run_bass_kernel_spmd(nc, in_maps, core_ids=list(range(NCORE)))
    out = np.concatenate([r["out"] for r in res.results], axis=0)
    return np.ascontiguousarray(out, np.float32)
